# revision 33
# baseline (speedup 1.0000x reference)
"""MRI data-consistency CG solver on 8 Trainium2 NeuronCores.

Sharding: pure data-parallel, 1 batch sample per core; the CG alpha/beta
scalars are global sums over the batch -> tiny AllReduce per CG round.

v2 design:
- Centered 2D (I)FFTs as chained PE matmuls of the folded [128, 3*320]
  field against the centered DFT matrix Fc (symmetric), alternating-
  transpose orientation (data is always lhsT) so no explicit transposes.
- Gauss 3-multiplication complex products: per stage only 3 real
  matmul-products (k1 = Xsum^T G1, k2 = Xi^T G2, k3 = Xr^T G3 with
  G1 = Gr, G2 = -(Gr+Gi), G3 = Gi-Gr), combined on the vector engines as
  Yr = k1+k2, Yi = k1+k3 (+ Ysum = Yr+Yi feeding the next stage's k1).
  For the IFFT (G = conj(Fc)) the same three constants serve with G2/G3
  swapped: only fr, fm = -(fr+fi), fd = fi-fr are kept on chip.
- bf16 coil pipeline (fields, csm, mask, DFT constants): PE rate is
  unchanged vs f32r, DVE elementwise gets the 2x 16-bit mode, csm DMA
  halves. CG state (p, r, b, q) and all alpha/beta math stay f32.
- Elementwise work is spread across DVE and GPSIMD by a greedy
  cost-weighted balancer; PSUM->SBUF combine reads split likewise.
- DMAs issue from the otherwise-idle SP sequencer.
- CG tail: ping-pong p so the b += alpha*p axpy can be deferred into the
  next iteration's first coils, off the serial path.

Folded field layout: block b (cols [b*320,(b+1)*320)) holds matrix rows
[b*128, ...); the junk region (partitions 64..127 of the last block) is
kept at exactly 0 so full-tile elementwise ops and reductions stay
correct.
"""

import numpy as np

CG_ITER = 10

_nc_cache = {}
LAST_RESULT = None


def _blocks(n):
    out = []
    r0 = 0
    while r0 < n:
        sz = min(128, n - r0)
        out.append((r0, sz))
        r0 += sz
    return out


def _centered_dft(n):
    # Columns of Fc = centered orthonormal DFT applied to unit vectors:
    # y = fftshift(fft(ifftshift(x))) = Fc @ x. Fc is symmetric for even n.
    eye = np.eye(n)
    Fc = np.fft.fftshift(
        np.fft.fft(np.fft.ifftshift(eye, axes=0), axis=0, norm="ortho"), axes=0
    )
    return Fc


def _build(Hc, Wc, Cc, iters, n_cores, no_collective=False):
    import concourse.bacc as bacc
    import concourse.mybir as mybir
    import concourse.tile as tile

    f32 = mybir.dt.float32
    bf16 = mybir.dt.bfloat16
    OP = mybir.AluOpType

    nc = bacc.Bacc(trn_type="TRN2", num_devices=n_cores)

    us = nc.dram_tensor("us_image", [2, Hc, Wc], f32, kind="ExternalInput")
    rec = nc.dram_tensor("reconstruction", [2, Hc, Wc], f32, kind="ExternalInput")
    mask_d = nc.dram_tensor("mask", [Hc, Wc], bf16, kind="ExternalInput")
    csm_r_d = nc.dram_tensor("csm_r", [Cc, Hc, Wc], bf16, kind="ExternalInput")
    csm_i_d = nc.dram_tensor("csm_i", [Cc, Hc, Wc], bf16, kind="ExternalInput")
    mu_d = nc.dram_tensor("mu", [1], f32, kind="ExternalInput")
    fr_d = nc.dram_tensor("f_r", [Hc, Hc], bf16, kind="ExternalInput")
    fm_d = nc.dram_tensor("f_m", [Hc, Hc], bf16, kind="ExternalInput")
    fd_d = nc.dram_tensor("f_d", [Hc, Hc], bf16, kind="ExternalInput")
    g1R_d = nc.dram_tensor("g1R", [2 * Hc, Hc], bf16, kind="ExternalInput")
    g1I_d = nc.dram_tensor("g1I", [2 * Hc, Hc], bf16, kind="ExternalInput")
    g3R_d = nc.dram_tensor("g3R", [2 * Hc, Hc], bf16, kind="ExternalInput")
    g3I_d = nc.dram_tensor("g3I", [2 * Hc, Hc], bf16, kind="ExternalInput")
    out_d = nc.dram_tensor("out", [2, Hc, Wc], f32, kind="ExternalOutput")

    BL = _blocks(Hc)
    NB = len(BL)
    FW = NB * Wc

    with tile.TileContext(nc) as tc:
        with (
            tc.tile_pool(name="consts", bufs=1) as consts,
            tc.tile_pool(name="state", bufs=1) as state,
            tc.tile_pool(name="work", bufs=1) as work,
            tc.tile_pool(name="small", bufs=1) as small,
            tc.tile_pool(name="psum", bufs=8, space="PSUM") as psp,
            tc.tile_pool(name="dram", bufs=4, space="DRAM") as dram,
        ):
            zero_me = []  # [128, FW] tiles that must start at 0

            def T(pool, name, shape, dtype=f32):
                tl = pool.tile(shape, dtype, tag=name)
                if list(shape) == [128, FW]:
                    zero_me.append(tl)
                return tl

            # constants
            fr = T(consts, "fr", [128, FW], bf16)
            fm = T(consts, "fm", [128, FW], bf16)
            fd = T(consts, "fd", [128, FW], bf16)
            g1R = T(consts, "g1R", [128, 2 * FW], bf16)
            g1I = T(consts, "g1I", [128, 2 * FW], bf16)
            g3R = T(consts, "g3R", [128, 2 * FW], bf16)
            g3I = T(consts, "g3I", [128, 2 * FW], bf16)
            maskf = T(consts, "maskf", [128, FW], bf16)
            ones_col = T(consts, "ones_col", [128, 1])
            ones_row = T(consts, "ones_row", [1, 128])
            mu_b = T(consts, "mu_b", [128, 1])
            mu_sb = T(consts, "mu_sb", [1, 1])

            # f32 CG state (p ping-pong)
            p_r = [T(state, f"p_r{j}", [128, FW]) for j in (0, 1)]
            p_i = [T(state, f"p_i{j}", [128, FW]) for j in (0, 1)]
            r_r = T(state, "r_r", [128, FW])
            r_i = T(state, "r_i", [128, FW])
            b_r = T(state, "b_r", [128, FW])
            b_i = T(state, "b_i", [128, FW])
            q_r = T(state, "q_r", [128, FW])
            q_i = T(state, "q_i", [128, FW])
            p16r = T(state, "p16r", [128, FW], bf16)
            p16i = T(state, "p16i", [128, FW], bf16)

            # bf16 coil pipeline, 2 slots
            def trio(nm):
                return [
                    [T(work, f"{nm}_{x}{j}", [128, FW], bf16) for x in "ris"]
                    for j in (0, 1)
                ]

            cs = [
                [T(work, f"cs_{x}{j}", [128, FW], bf16) for x in "ri"]
                for j in (0, 1, 2, 3)
            ]
            cp = trio("cp")
            km = trio("km")
            zz = [[T(work, f"zz_{x}{j}", [128, FW], bf16) for x in "ri"] for j in (0, 1)]
            # stacked [Xr; Xi] fields for the K-stacked 4-mult stages:
            # blocks: [Xr full-128 blocks, Xi full-128 blocks, (Xr tail | Xi tail)]
            NBF = Hc // 128          # full 128-row blocks per component
            REM = Hc - NBF * 128     # tail rows (64 for Hc=320, 0 for 128)
            NBS = 2 * NBF + (1 if REM else 0)
            SW = NBS * Wc
            st1 = [T(work, f"st1_{j}", [128, SW], bf16) for j in (0, 1)]
            st3 = [T(work, f"st3_{j}", [128, SW], bf16) for j in (0, 1)]
            tshift = [T(work, f"tshift_{j}", [128, Wc], bf16) for j in (0, 1)]
            sc16 = [
                [T(work, f"sc16_{k}{j}", [128, FW], bf16) for k in range(4)]
                for j in (0, 1)
            ]
            ac16 = [
                [T(work, f"ac16_{k}{j}", [128, FW], bf16) for k in range(4)]
                for j in (0, 1)
            ]
            # bf16 scratch for reduction products (cheap bf16 reduces)
            prod16 = [T(work, f"prod16_{k}", [128, FW], bf16) for k in range(4)]
            # bf16 staging for Gauss products (Act evacuates PSUM banks here)
            ksb16 = [
                [T(work, f"ksb16_{k}{j}", [128, FW], bf16) for k in range(3)]
                for j in (0, 1)
            ]

            partials = T(small, "partials", [128, 16])
            redsums = T(small, "redsums", [1, 16])
            asum_t = T(small, "asum", [1, 16])
            scl = T(small, "scl", [1, 16])
            alphas = T(small, "alphas", [1, 8])
            bc = T(small, "bc", [128, 8])
            rr_t = T(small, "rr", [1, 1])
            rrn_t = T(small, "rrn", [1, 1])

            v = nc.vector
            g = nc.gpsimd
            a = nc.scalar
            sp = nc.sync
            STT_OP = "scalar_tensor_tensor"

            # engines are pinned per op class:
            #   DVE: all bf16 elementwise, STT axpys, reduces (fast, flexible)
            #   Pool (GPSIMD): only the q-accumulate f32 adds and a few tail
            #     products -- its flat ~2us/op cost is tolerable off the
            #     critical path (no PSUM access, no TensorScalarPtr there)
            #   Act: all PSUM evacuation copies (fp32 bank -> bf16 SBUF)
            def TTv(out, in0, in1, op):
                v.tensor_tensor(out=out, in0=in0, in1=in1, op=op)

            def TTg(out, in0, in1, op):
                g.tensor_tensor(out=out, in0=in0, in1=in1, op=op)

            def STT(out, in0, scalar, in1, op0, op1):
                v.scalar_tensor_tensor(
                    out=out, in0=in0, scalar=scalar, in1=in1, op0=op0, op1=op1
                )

            def RSUM(out, in_, col):
                v.reduce_sum(out=out[:, col : col + 1], in_=in_,
                             axis=mybir.AxisListType.X)

            # ---------- init ----------

            def load_folded(dst, src2d):
                nbf = Hc // 128
                full = nbf * 128
                if nbf:
                    sp.dma_start(
                        out=dst[:, 0 : nbf * Wc].rearrange("p (b w) -> p b w", b=nbf),
                        in_=src2d[0:full, :].rearrange("(b p) w -> p b w", p=128),
                    )
                if full < Hc:
                    rem = Hc - full
                    sp.dma_start(
                        out=dst[:rem, nbf * Wc : (nbf + 1) * Wc],
                        in_=src2d[full:Hc, :],
                    )

            def store_folded(src, dst2d):
                nbf = Hc // 128
                full = nbf * 128
                if nbf:
                    sp.dma_start(
                        out=dst2d[0:full, :].rearrange("(b p) w -> p b w", p=128),
                        in_=src[:, 0 : nbf * Wc].rearrange("p (b w) -> p b w", b=nbf),
                    )
                if full < Hc:
                    rem = Hc - full
                    sp.dma_start(
                        out=dst2d[full:Hc, :],
                        in_=src[:rem, nbf * Wc : (nbf + 1) * Wc],
                    )

            # 1) zero the junk regions of DMA-target tiles first
            pre = [maskf] + [t for pairt in cs for t in pairt]
            for tl in pre:
                v.memset(tl, 0.0)
            # 2) issue constant + first csm loads (SP) while zeroing the rest
            load_folded(fr, fr_d[:])
            load_folded(fm, fm_d[:])
            load_folded(fd, fd_d[:])
            for gt, gd in ((g1R, g1R_d), (g1I, g1I_d), (g3R, g3R_d),
                           (g3I, g3I_d)):
                sp.dma_start(
                    out=gt[:, : NBS * Wc].rearrange("p (b w) -> p b w", b=NBS),
                    in_=gd[: NBS * 128, :].rearrange("(b p) w -> p b w", p=128),
                )
            load_folded(maskf, mask_d[:])
            sp.dma_start(out=mu_sb[:1, :1], in_=mu_d[None, :])

            def load_csm(ci_, slot):
                load_folded(cs[slot][0], csm_r_d[ci_])
                load_folded(cs[slot][1], csm_i_d[ci_])

            load_csm(0, 0)
            if Cc > 1:
                load_csm(1, 1)
            # fr/fm/fd junk is only ever seen by matmul rhs chunk APs
            # (never read) -- and they are already loaded: do NOT zero them.
            preset = set(id(t) for t in (
                [maskf, fr, fm, fd] + [t for pairt in cs for t in pairt]))
            for tl in zero_me:
                if id(tl) in preset:
                    continue
                if tl.dtype == bf16:
                    v.memset(tl, 0.0)
                else:
                    g.memset(tl, 0.0)
            v.memset(partials, 0.0)
            v.memset(ones_col, 1.0)
            v.memset(ones_row, 1.0)

            # r = us + mu*rec; p0 = r; p16 = bf16(r); b = 0
            # stage us/rec through tiles that are overwritten in iter 0
            load_folded(p_r[1], us[0])
            load_folded(p_i[1], us[1])
            load_folded(q_r, rec[0])
            load_folded(q_i, rec[1])
            psb = psp.tile([128, 16], f32, tag="mm")
            nc.tensor.matmul(
                psb[:, :1], lhsT=ones_row[:1, :128], rhs=mu_sb[:1, :1],
                start=True, stop=True,
            )
            a.copy(out=mu_b[:, :1], in_=psb[:, :1])
            v.scalar_tensor_tensor(out=r_r, in0=q_r, scalar=mu_b[:, :1],
                                   in1=p_r[1], op0=OP.mult, op1=OP.add)
            v.scalar_tensor_tensor(out=r_i, in0=q_i, scalar=mu_b[:, :1],
                                   in1=p_i[1], op0=OP.mult, op1=OP.add)
            a.copy(out=p_r[0], in_=r_r)
            a.copy(out=p_i[0], in_=r_i)
            a.copy(out=p16r, in_=r_r)
            a.copy(out=p16i, in_=r_i)


            def gauss_stage(xr, xi, xs, g1, g2, g3, consume):
                # complex product (xr + i*xi)^T (Gr + i*Gi) via 3 real products:
                # k1 = xs^T g1, k2 = xi^T g2, k3 = xr^T g3
                # Yr = k1 + k2 ; Yi = k1 + k3
                m_order = ([NB - 1] + list(range(NB - 1))) if REM else range(NB)
                for m in m_order:
                    m0, msz = BL[m]
                    k1t = psp.tile([128, Wc], f32, tag="mm")
                    k2t = psp.tile([128, Wc], f32, tag="mm")
                    k3t = psp.tile([128, Wc], f32, tag="mm")
                    # emit k3 first: its input (xr) is ready earliest
                    for bank, (srcd, gg) in ((k3t, (xr, g3)), (k2t, (xi, g2)),
                                             (k1t, (xs, g1))):
                        for k, (k0, ksz) in enumerate(BL):
                            nc.tensor.matmul(
                                bank[:msz, :],
                                lhsT=srcd[:ksz, k * Wc + m0 : k * Wc + m0 + msz],
                                rhs=gg[:ksz, k * Wc : (k + 1) * Wc],
                                start=(k == 0), stop=(k == NB - 1),
                            )
                    consume(m, msz, k1t, k2t, k3t)

            def fourmult_stacked(st, gRst, gIst, consume):
                # K-stacked complex product: the [Xr; Xi] stacking makes the
                # 2K contraction exactly NBS chunks of 128 -- 2 products
                # instead of 4, no padding waste.
                for m, (m0, msz) in enumerate(BL):
                    bR = psp.tile([128, Wc], f32, tag="mm")
                    bI = psp.tile([128, Wc], f32, tag="mm")
                    for bank, gg in ((bR, gRst), (bI, gIst)):
                        for k in range(NBS):
                            nc.tensor.matmul(
                                bank[:msz, :],
                                lhsT=st[:, k * Wc + m0 : k * Wc + m0 + msz],
                                rhs=gg[:, k * Wc : (k + 1) * Wc],
                                start=(k == 0), stop=(k == NBS - 1),
                            )
                    consume(m, msz, bR, bI)

            def evac_copy(dst_r, dst_i):
                def f(m, msz, bR, bI):
                    sl = slice(m * Wc, (m + 1) * Wc)
                    a.copy(out=dst_r[:msz, sl], in_=bR[:msz, :])
                    a.copy(out=dst_i[:msz, sl], in_=bI[:msz, :])
                return f

            def evac_stage_banks(slot):
                # Act copies each Gauss PSUM bank to bf16 staging per m-block
                k1s, k2s, k3s = ksb16[slot]

                def f(m, msz, k1, k2, k3):
                    sl = slice(m * Wc, (m + 1) * Wc)
                    a.copy(out=k1s[:msz, sl], in_=k1[:msz, :])
                    a.copy(out=k2s[:msz, sl], in_=k2[:msz, :])
                    a.copy(out=k3s[:msz, sl], in_=k3[:msz, :])
                return f

            def gauss_tail_combine(slot, st):
                # remainder rows: Yr tail -> stacked tail block p[0:REM];
                # Yi tail -> scratch, then SP-DMA partition-shift to p[REM:]
                if not REM:
                    return
                k1s, k2s, k3s = ksb16[slot]
                csl = slice(NBF * Wc, NBF * Wc + Wc)
                tb = (NBS - 1) * Wc
                TTv(st[:REM, tb : tb + Wc], k1s[:REM, csl], k2s[:REM, csl],
                    OP.add)
                scr = tshift[slot]
                TTv(scr[:REM, :], k1s[:REM, csl], k3s[:REM, csl], OP.add)
                sp.dma_start(out=st[REM : 2 * REM, tb : tb + Wc],
                             in_=scr[:REM, :])

            def gauss_main_combine(slot, st):
                # full blocks: Yr -> stacked blocks [0, NBF), Yi -> [NBF, 2NBF)
                k1s, k2s, k3s = ksb16[slot]
                if NBF:
                    w = NBF * Wc
                    TTv(st[:, 0:w], k1s[:, 0:w], k2s[:, 0:w], OP.add)
                    TTv(st[:, w : 2 * w], k1s[:, 0:w], k3s[:, 0:w], OP.add)

            deferred = []  # per-iteration deferred ops (b axpy), emitted in coils

            def proj_coil(c):
                slot = c % 2
                csr, csi = cs[c % 4]
                cpr, cpi, cps = cp[slot]
                A16, B16, C16, D16 = sc16[slot]
                # projection cp = p * csm (all bf16, DVE)
                TTv(A16, p16r, csr, OP.mult)
                TTv(B16, p16i, csi, OP.mult)
                TTv(cpr, A16, B16, OP.subtract)
                TTv(C16, p16r, csi, OP.mult)
                TTv(D16, p16i, csr, OP.mult)
                TTv(cpi, C16, D16, OP.add)
                TTv(cps, cpr, cpi, OP.add)

            def stage_coil(c, s):
                slot = c % 2
                if s == 0:
                    # FFT rows: Gauss; banks staged by Act; stacked output
                    bk = evac_stage_banks(slot)

                    def con0(m, msz, k1, k2, k3):
                        bk(m, msz, k1, k2, k3)
                        if REM and m == NB - 1:
                            gauss_tail_combine(slot, st1[slot])
                    gauss_stage(*cp[slot], fr, fm, fd, con0)
                    if not REM:
                        gauss_tail_combine(slot, st1[slot])
                    gauss_main_combine(slot, st1[slot])
                elif s == 1:
                    # FFT cols: stacked 4-mult, Act evac, then mask
                    A16, B16 = sc16[slot][0], sc16[slot][1]
                    fourmult_stacked(st1[slot], g1R, g1I, evac_copy(A16, B16))
                    kr, ki, ksm = km[slot]
                    TTv(kr, A16, maskf, OP.mult)
                    TTv(ki, B16, maskf, OP.mult)
                    TTv(ksm, kr, ki, OP.add)
                elif s == 2:
                    # IFFT rows: Gauss (G2/G3 swapped); stacked output
                    bk = evac_stage_banks(slot)

                    def con2(m, msz, k1, k2, k3):
                        bk(m, msz, k1, k2, k3)
                        if REM and m == NB - 1:
                            gauss_tail_combine(slot, st3[slot])
                    gauss_stage(*km[slot], fr, fd, fm, con2)
                    if not REM:
                        gauss_tail_combine(slot, st3[slot])
                    gauss_main_combine(slot, st3[slot])
                else:
                    # IFFT cols: stacked 4-mult straight into z
                    fourmult_stacked(st3[slot], g3R, g3I, evac_copy(*zz[slot]))

            def accum_coil(c, last=0):
                slot = c % 2
                csr, csi = cs[c % 4]
                A16, B16, C16, D16 = ac16[slot]
                zr, zi = zz[slot]
                # q += z * conj(csm): products on DVE; accumulates on Pool
                # normally (hidden under PE). For the tail-adjacent last pair
                # (last=1: first coil, last=2: final coil) split engines so
                # the final q is ready sooner for the reduction dots.
                TTv(A16, zr, csr, OP.mult)
                (TTv if last else TTg)(q_r, q_r, A16, OP.add)
                TTv(B16, zi, csi, OP.mult)
                (TTv if last else TTg)(q_r, q_r, B16, OP.add)
                TTv(C16, zi, csr, OP.mult)
                TTg(q_i, q_i, C16, OP.add)
                TTv(D16, zr, csi, OP.mult)
                TTg(q_i, q_i, D16, OP.subtract)
                # slip one deferred op from the previous iteration's tail in
                if deferred:
                    deferred.pop(0)()

            def coil_pair(it, c0):
                # 2-coil software pipeline: interleave the two coils' stages
                # so one coil's PE matmuls cover the other's evacuation.
                # proj for THIS pair was already emitted by the previous pair
                # (or the iteration prologue); emit the NEXT pair's proj
                # before this pair's q-accumulate so the next pair's first
                # matmuls are never blocked behind accum on DVE.
                pair = [c0] + ([c0 + 1] if c0 + 1 < Cc else [])
                # prefetch the next pair's csm (2 ahead)
                for cn in (c0 + 2, c0 + 3):
                    if cn < Cc:
                        load_csm(cn, cn % 4)
                    elif it + 1 < iters and cn - Cc in (0, 1):
                        load_csm(cn - Cc, (cn - Cc) % 4)
                for s in range(4):
                    for c in pair:
                        stage_coil(c, s)
                for cn in (c0 + 2, c0 + 3):
                    if cn < Cc:
                        proj_coil(cn)
                is_last_pair = c0 + 2 >= Cc
                for j, c in enumerate(pair):
                    accum_coil(c, last=(j + 1 if is_last_pair else 0))

            def reduction_round(k):
                ps1 = psp.tile([1, 16], f32, tag="mm")
                nc.tensor.matmul(ps1[:1, :k], lhsT=ones_col[:, :1],
                                 rhs=partials[:, :k], start=True, stop=True)
                a.copy(out=redsums[:1, :k], in_=ps1[:1, :k])
                din = dram.tile([1, 16], f32, tag="cin")
                dout = dram.tile([1, 16], f32, tag="cout")
                sp.dma_start(out=din[:1, :k], in_=redsums[:1, :k])
                if n_cores > 1 and not no_collective:
                    nc.gpsimd.collective_compute(
                        "AllReduce", OP.add,
                        replica_groups=[list(range(n_cores))],
                        ins=[din[:1, :k].opt()],
                        outs=[dout[:1, :k].opt()],
                    )
                else:
                    sp.dma_start(out=dout[:1, :k], in_=din[:1, :k])
                sp.dma_start(out=asum_t[:1, :k], in_=dout[:1, :k])
                return asum_t

            def dotcol(x, y, col, eng="v"):
                # partials[:, col] = rowsum(bf16(x * y))
                j = dotcol.j
                dotcol.j = (j + 1) % 4
                t = prod16[j]
                (TTv if eng == "v" else TTg)(t, x, y, OP.mult)
                RSUM(partials, t, col)
            dotcol.j = 0

            for it in range(iters):
                pcur = it % 2
                pnew = (it + 1) % 2
                pr_, pi_ = p_r[pcur], p_i[pcur]
                # q = mu * p (coils accumulate on top)
                proj_coil(0)
                if Cc > 1:
                    proj_coil(1)
                # q-init and the hoisted (r,r) dots are not needed until the
                # first accumulate -- emit them behind the projections so the
                # first pair's matmuls start as early as possible
                v.tensor_scalar_mul(out=q_r, in0=pr_, scalar1=mu_b[:, :1])
                v.tensor_scalar_mul(out=q_i, in0=pi_, scalar1=mu_b[:, :1])
                if it == 0:
                    dotcol(r_r, r_r, 10, "g")
                    dotcol(r_i, r_i, 11, "v")
                for c0 in range(0, Cc, 2):
                    coil_pair(it, c0)
                # ---- merged reduction round:
                #   pq = sum(q conj(p)); t = sum(q conj(r)); qq = sum(|q|^2)
                #   rr_new = rr - 2 Re(conj(alpha) t) + |alpha|^2 qq
                dotcol(q_r, pr_, 0, "v")
                dotcol(q_r, pi_, 3, "v")
                dotcol(q_r, r_r, 4, "v")
                dotcol(q_r, r_i, 7, "g")
                dotcol(q_r, q_r, 8, "v")
                dotcol(q_i, pi_, 1, "g")
                dotcol(q_i, pr_, 2, "g")
                dotcol(q_i, r_i, 5, "g")
                dotcol(q_i, r_r, 6, "g")
                dotcol(q_i, q_i, 9, "v")
                k = 12 if it == 0 else 10
                asum = reduction_round(k)
                TTv(out=scl[:1, 0:1], in0=asum[:1, 0:1], in1=asum[:1, 1:2],
                    op=OP.add)       # pq_r
                TTv(out=scl[:1, 1:2], in0=asum[:1, 2:3], in1=asum[:1, 3:4],
                    op=OP.subtract)  # pq_i
                TTv(out=scl[:1, 6:7], in0=asum[:1, 4:5], in1=asum[:1, 5:6],
                    op=OP.add)       # t_r
                TTv(out=scl[:1, 7:8], in0=asum[:1, 6:7], in1=asum[:1, 7:8],
                    op=OP.subtract)  # t_i
                TTv(out=scl[:1, 8:9], in0=asum[:1, 8:9], in1=asum[:1, 9:10],
                    op=OP.add)       # qq
                if it == 0:
                    TTv(out=rr_t[:1, :1], in0=asum[:1, 10:11],
                        in1=asum[:1, 11:12], op=OP.add)
                TTv(out=scl[:1, 2:3], in0=scl[:1, 0:1], in1=scl[:1, 0:1],
                    op=OP.mult)
                TTv(out=scl[:1, 3:4], in0=scl[:1, 1:2], in1=scl[:1, 1:2],
                    op=OP.mult)
                TTv(out=scl[:1, 2:3], in0=scl[:1, 2:3], in1=scl[:1, 3:4],
                    op=OP.add)       # |pq|^2
                v.reciprocal(out=scl[:1, 5:6], in_=scl[:1, 2:3])
                TTv(out=scl[:1, 4:5], in0=rr_t[:1, :1], in1=scl[:1, 5:6],
                    op=OP.mult)      # g = rr/|pq|^2
                # alphas: [a_r, na_i, na_r, a_i, beta]; alpha = g*conj(pq)
                TTv(out=alphas[:1, 0:1], in0=scl[:1, 4:5], in1=scl[:1, 0:1],
                    op=OP.mult)
                TTv(out=alphas[:1, 1:2], in0=scl[:1, 4:5], in1=scl[:1, 1:2],
                    op=OP.mult)
                v.tensor_scalar_mul(out=alphas[:1, 2:3], in0=alphas[:1, 0:1],
                                    scalar1=-1.0)
                v.tensor_scalar_mul(out=alphas[:1, 3:4], in0=alphas[:1, 1:2],
                                    scalar1=-1.0)
                # broadcast alpha immediately: the r-updates only need it
                v.reciprocal(out=scl[:1, 14:15], in_=rr_t[:1, :1])
                psbA = psp.tile([128, 16], f32, tag="mm")
                nc.tensor.matmul(psbA[:, :4], lhsT=ones_row[:1, :128],
                                 rhs=alphas[:1, :4], start=True, stop=True)
                a.copy(out=bc[:, :4], in_=psbA[:, :4])
                a_r = bc[:, 0:1]
                na_i = bc[:, 1:2]
                na_r = bc[:, 2:3]
                a_i = bc[:, 3:4]
                bet = bc[:, 4:5]
                # critical path: r -= alpha*q on DVE, while the beta chain
                # (rr_new expansion) runs concurrently on GPSIMD
                v.scalar_tensor_tensor(out=r_r, in0=q_r, scalar=na_r, in1=r_r,
                                       op0=OP.mult, op1=OP.add)
                TTg(scl[:1, 9:10], alphas[:1, 0:1], scl[:1, 6:7], OP.mult)
                TTg(scl[:1, 10:11], alphas[:1, 1:2], scl[:1, 7:8], OP.mult)
                TTg(scl[:1, 9:10], scl[:1, 9:10], scl[:1, 10:11], OP.add)
                TTg(scl[:1, 11:12], alphas[:1, 0:1], alphas[:1, 0:1], OP.mult)
                TTg(scl[:1, 12:13], alphas[:1, 1:2], alphas[:1, 1:2], OP.mult)
                TTg(scl[:1, 11:12], scl[:1, 11:12], scl[:1, 12:13], OP.add)
                TTg(scl[:1, 12:13], scl[:1, 11:12], scl[:1, 8:9], OP.mult)
                TTg(scl[:1, 10:11], scl[:1, 9:10], scl[:1, 9:10], OP.add)
                TTg(scl[:1, 13:14], rr_t[:1, :1], scl[:1, 10:11], OP.subtract)
                TTg(rrn_t[:1, :1], scl[:1, 13:14], scl[:1, 12:13], OP.add)
                TTg(alphas[:1, 4:5], rrn_t[:1, :1], scl[:1, 14:15], OP.mult)
                psbB = psp.tile([128, 16], f32, tag="mm")
                nc.tensor.matmul(psbB[:, :1], lhsT=ones_row[:1, :128],
                                 rhs=alphas[:1, 4:5], start=True, stop=True)
                a.copy(out=bc[:, 4:5], in_=psbB[:, :1])
                a.copy(out=rr_t[:1, :1], in_=rrn_t[:1, :1])
                v.scalar_tensor_tensor(out=r_i, in0=q_i, scalar=na_r, in1=r_i,
                                       op0=OP.mult, op1=OP.add)
                v.scalar_tensor_tensor(out=r_r, in0=q_i, scalar=a_i, in1=r_r,
                                       op0=OP.mult, op1=OP.add)
                v.scalar_tensor_tensor(out=r_i, in0=q_r, scalar=na_i, in1=r_i,
                                       op0=OP.mult, op1=OP.add)
                v.scalar_tensor_tensor(out=p_r[pnew], in0=pr_, scalar=bet,
                                       in1=r_r, op0=OP.mult, op1=OP.add)
                v.scalar_tensor_tensor(out=p_i[pnew], in0=pi_, scalar=bet,
                                       in1=r_i, op0=OP.mult, op1=OP.add)
                a.copy(out=p16r, in_=p_r[pnew])
                a.copy(out=p16i, in_=p_i[pnew])

                # b += alpha*p (old p) -- deferred into next iteration's coils
                def mk(eng, out, in0, sca, in1):
                    def run():
                        getattr(eng, STT_OP)(out=out, in0=in0, scalar=sca,
                                             in1=in1, op0=OP.mult, op1=OP.add)
                    return run

                dops = [
                    mk(v, b_r, pr_, a_r, b_r),
                    mk(v, b_i, pi_, a_r, b_i),
                    mk(v, b_r, pi_, na_i, b_r),
                    mk(v, b_i, pr_, a_i, b_i),
                ]
                if it + 1 < iters:
                    deferred.extend(dops)
                else:
                    for d in dops:
                        d()

            import os as _os
            if _os.environ.get("KDBG") == "q":
                store_folded(q_r, out_d[0])
                store_folded(q_i, out_d[1])
            elif _os.environ.get("KDBG") == "z":
                zlast = zz[(Cc - 1) % 2]
                v.tensor_scalar_mul(out=r_r, in0=zlast[0], scalar1=1.0)
                v.tensor_scalar_mul(out=r_i, in0=zlast[1], scalar1=1.0)
                store_folded(r_r, out_d[0])
                store_folded(r_i, out_d[1])
            elif _os.environ.get("KDBG") == "km":
                klast = km[(Cc - 1) % 2]
                v.tensor_scalar_mul(out=r_r, in0=klast[0], scalar1=1.0)
                v.tensor_scalar_mul(out=r_i, in0=klast[1], scalar1=1.0)
                store_folded(r_r, out_d[0])
                store_folded(r_i, out_d[1])
            elif _os.environ.get("KDBG") == "s1":
                stl = st1[(Cc - 1) % 2]
                wv = NBF * Wc
                # Yr: full blocks then tail block lower half
                v.tensor_scalar_mul(out=r_r[:, 0:wv], in0=stl[:, 0:wv],
                                    scalar1=1.0)
                v.tensor_scalar_mul(out=r_i[:, 0:wv], in0=stl[:, wv : 2 * wv],
                                    scalar1=1.0)
                if REM:
                    tbv = (NBS - 1) * Wc
                    v.tensor_scalar_mul(
                        out=r_r[:REM, NBF * Wc : NBF * Wc + Wc],
                        in0=stl[:REM, tbv : tbv + Wc], scalar1=1.0)
                    v.tensor_scalar_mul(
                        out=r_i[:REM, NBF * Wc : NBF * Wc + Wc],
                        in0=stl[REM : 2 * REM, tbv : tbv + Wc], scalar1=1.0)
                store_folded(r_r, out_d[0])
                store_folded(r_i, out_d[1])
            elif _os.environ.get("KDBG") == "p0":
                store_folded(p_r[0], out_d[0])
                store_folded(p_i[0], out_d[1])
            elif _os.environ.get("KDBG") == "mub":
                nc.scalar.copy(out=r_r[:, 0:1], in_=mu_b[:, 0:1])
                store_folded(r_r, out_d[0])
                store_folded(r_i, out_d[1])
            elif _os.environ.get("KDBG") == "cp":
                clast = cp[(Cc - 1) % 2]
                v.tensor_scalar_mul(out=r_r, in0=clast[0], scalar1=1.0)
                v.tensor_scalar_mul(out=r_i, in0=clast[1], scalar1=1.0)
                store_folded(r_r, out_d[0])
                store_folded(r_i, out_d[1])
            else:
                store_folded(b_r, out_d[0])
                store_folded(b_i, out_d[1])

    nc.compile()
    return nc


def _stack_g(GA, GB, Hc):
    # row order must match the on-chip stacked layout: full 128-row blocks of
    # A, then of B, then the interleaved tail block [A-tail; B-tail]
    full = (Hc // 128) * 128
    return np.concatenate([GA[:full], GB[:full], GA[full:], GB[full:]], axis=0)


def _prep_consts(Hc):
    import ml_dtypes

    bf = ml_dtypes.bfloat16
    Fc = _centered_dft(Hc)
    fr = np.ascontiguousarray(Fc.real).astype(np.float32)
    fi = np.ascontiguousarray(Fc.imag).astype(np.float32)
    fni = -fi
    return {
        "f_r": fr.astype(bf),
        "f_m": (-(fr + fi)).astype(bf),
        "f_d": (fi - fr).astype(bf),
        # stage 1 (FFT):  Yr = Xr^T fr + Xi^T (-fi); Yi = Xr^T fi + Xi^T fr
        "g1R": _stack_g(fr, fni, Hc).astype(bf),
        "g1I": _stack_g(fi, fr, Hc).astype(bf),
        # stage 3 (IFFT): Yr = Xr^T fr + Xi^T fi;    Yi = Xr^T (-fi) + Xi^T fr
        "g3R": _stack_g(fr, fi, Hc).astype(bf),
        "g3I": _stack_g(fni, fr, Hc).astype(bf),
    }


def kernel(us_image, reconstruction, mask, csm_r, csm_i, mu):
    global LAST_RESULT
    import ml_dtypes
    from concourse.bass_utils import run_bass_kernel_spmd

    bf = ml_dtypes.bfloat16
    Bc, _, Hc, Wc = us_image.shape
    Cc = csm_r.shape[1]
    n_cores = Bc
    iters = CG_ITER

    key = (Hc, Wc, Cc, iters, n_cores)
    if key not in _nc_cache:
        _nc_cache[key] = _build(Hc, Wc, Cc, iters, n_cores)
    nc = _nc_cache[key]

    gconsts = _prep_consts(Hc)

    in_maps = []
    for b in range(n_cores):
        in_maps.append(
            {
                "us_image": np.ascontiguousarray(us_image[b], dtype=np.float32),
                "reconstruction": np.ascontiguousarray(
                    reconstruction[b], dtype=np.float32
                ),
                "mask": np.ascontiguousarray(mask[b, 0]).astype(bf),
                "csm_r": np.ascontiguousarray(csm_r[b]).astype(bf),
                "csm_i": np.ascontiguousarray(csm_i[b]).astype(bf),
                "mu": np.ascontiguousarray(mu, dtype=np.float32),
                **gconsts,
            }
        )

    res = run_bass_kernel_spmd(nc, in_maps, core_ids=list(range(n_cores)))
    LAST_RESULT = res
    out = np.stack([res.results[b]["out"] for b in range(n_cores)], axis=0)
    return out.astype(np.float32)


# revision 34
# speedup vs baseline: 1.0031x; 1.0031x over previous
"""MRI data-consistency CG solver on 8 Trainium2 NeuronCores.

Sharding: pure data-parallel, 1 batch sample per core; the CG alpha/beta
scalars are global sums over the batch -> tiny AllReduce per CG round.

v2 design:
- Centered 2D (I)FFTs as chained PE matmuls of the folded [128, 3*320]
  field against the centered DFT matrix Fc (symmetric), alternating-
  transpose orientation (data is always lhsT) so no explicit transposes.
- Gauss 3-multiplication complex products: per stage only 3 real
  matmul-products (k1 = Xsum^T G1, k2 = Xi^T G2, k3 = Xr^T G3 with
  G1 = Gr, G2 = -(Gr+Gi), G3 = Gi-Gr), combined on the vector engines as
  Yr = k1+k2, Yi = k1+k3 (+ Ysum = Yr+Yi feeding the next stage's k1).
  For the IFFT (G = conj(Fc)) the same three constants serve with G2/G3
  swapped: only fr, fm = -(fr+fi), fd = fi-fr are kept on chip.
- bf16 coil pipeline (fields, csm, mask, DFT constants): PE rate is
  unchanged vs f32r, DVE elementwise gets the 2x 16-bit mode, csm DMA
  halves. CG state (p, r, b, q) and all alpha/beta math stay f32.
- Elementwise work is spread across DVE and GPSIMD by a greedy
  cost-weighted balancer; PSUM->SBUF combine reads split likewise.
- DMAs issue from the otherwise-idle SP sequencer.
- CG tail: ping-pong p so the b += alpha*p axpy can be deferred into the
  next iteration's first coils, off the serial path.

Folded field layout: block b (cols [b*320,(b+1)*320)) holds matrix rows
[b*128, ...); the junk region (partitions 64..127 of the last block) is
kept at exactly 0 so full-tile elementwise ops and reductions stay
correct.
"""

import numpy as np

CG_ITER = 10

_nc_cache = {}
LAST_RESULT = None


def _blocks(n):
    out = []
    r0 = 0
    while r0 < n:
        sz = min(128, n - r0)
        out.append((r0, sz))
        r0 += sz
    return out


def _centered_dft(n):
    # Columns of Fc = centered orthonormal DFT applied to unit vectors:
    # y = fftshift(fft(ifftshift(x))) = Fc @ x. Fc is symmetric for even n.
    eye = np.eye(n)
    Fc = np.fft.fftshift(
        np.fft.fft(np.fft.ifftshift(eye, axes=0), axis=0, norm="ortho"), axes=0
    )
    return Fc


def _build(Hc, Wc, Cc, iters, n_cores, no_collective=False):
    import concourse.bacc as bacc
    import concourse.mybir as mybir
    import concourse.tile as tile

    f32 = mybir.dt.float32
    bf16 = mybir.dt.bfloat16
    OP = mybir.AluOpType

    nc = bacc.Bacc(trn_type="TRN2", num_devices=n_cores)

    us = nc.dram_tensor("us_image", [2, Hc, Wc], f32, kind="ExternalInput")
    rec = nc.dram_tensor("reconstruction", [2, Hc, Wc], f32, kind="ExternalInput")
    mask_d = nc.dram_tensor("mask", [Hc, Wc], bf16, kind="ExternalInput")
    csm_r_d = nc.dram_tensor("csm_r", [Cc, Hc, Wc], bf16, kind="ExternalInput")
    csm_i_d = nc.dram_tensor("csm_i", [Cc, Hc, Wc], bf16, kind="ExternalInput")
    mu_d = nc.dram_tensor("mu", [1], f32, kind="ExternalInput")
    fr_d = nc.dram_tensor("f_r", [Hc, Hc], bf16, kind="ExternalInput")
    fm_d = nc.dram_tensor("f_m", [Hc, Hc], bf16, kind="ExternalInput")
    fd_d = nc.dram_tensor("f_d", [Hc, Hc], bf16, kind="ExternalInput")
    g1R_d = nc.dram_tensor("g1R", [2 * Hc, Hc], bf16, kind="ExternalInput")
    g1I_d = nc.dram_tensor("g1I", [2 * Hc, Hc], bf16, kind="ExternalInput")
    g3R_d = nc.dram_tensor("g3R", [2 * Hc, Hc], bf16, kind="ExternalInput")
    g3I_d = nc.dram_tensor("g3I", [2 * Hc, Hc], bf16, kind="ExternalInput")
    out_d = nc.dram_tensor("out", [2, Hc, Wc], f32, kind="ExternalOutput")

    BL = _blocks(Hc)
    NB = len(BL)
    FW = NB * Wc

    with tile.TileContext(nc) as tc:
        with (
            tc.tile_pool(name="consts", bufs=1) as consts,
            tc.tile_pool(name="state", bufs=1) as state,
            tc.tile_pool(name="work", bufs=1) as work,
            tc.tile_pool(name="small", bufs=1) as small,
            tc.tile_pool(name="psum", bufs=8, space="PSUM") as psp,
            tc.tile_pool(name="dram", bufs=4, space="DRAM") as dram,
        ):
            zero_me = []  # [128, FW] tiles that must start at 0

            def T(pool, name, shape, dtype=f32):
                tl = pool.tile(shape, dtype, tag=name)
                if list(shape) == [128, FW]:
                    zero_me.append(tl)
                return tl

            # constants
            fr = T(consts, "fr", [128, FW], bf16)
            fm = T(consts, "fm", [128, FW], bf16)
            fd = T(consts, "fd", [128, FW], bf16)
            g1R = T(consts, "g1R", [128, 2 * FW], bf16)
            g1I = T(consts, "g1I", [128, 2 * FW], bf16)
            g3R = T(consts, "g3R", [128, 2 * FW], bf16)
            g3I = T(consts, "g3I", [128, 2 * FW], bf16)
            maskf = T(consts, "maskf", [128, FW], bf16)
            ones_col = T(consts, "ones_col", [128, 1])
            ones_row = T(consts, "ones_row", [1, 128])
            mu_b = T(consts, "mu_b", [128, 1])
            mu_sb = T(consts, "mu_sb", [1, 1])

            # f32 CG state (p ping-pong)
            p_r = [T(state, f"p_r{j}", [128, FW]) for j in (0, 1)]
            p_i = [T(state, f"p_i{j}", [128, FW]) for j in (0, 1)]
            r_r = T(state, "r_r", [128, FW])
            r_i = T(state, "r_i", [128, FW])
            b_r = T(state, "b_r", [128, FW])
            b_i = T(state, "b_i", [128, FW])
            q_r = T(state, "q_r", [128, FW])
            q_i = T(state, "q_i", [128, FW])
            p16r = T(state, "p16r", [128, FW], bf16)
            p16i = T(state, "p16i", [128, FW], bf16)

            # bf16 coil pipeline, 2 slots
            def trio(nm):
                return [
                    [T(work, f"{nm}_{x}{j}", [128, FW], bf16) for x in "ris"]
                    for j in (0, 1)
                ]

            cs = [
                [T(work, f"cs_{x}{j}", [128, FW], bf16) for x in "ri"]
                for j in (0, 1, 2, 3)
            ]
            cp = trio("cp")
            km = trio("km")
            zz = [[T(work, f"zz_{x}{j}", [128, FW], bf16) for x in "ri"] for j in (0, 1)]
            # stacked [Xr; Xi] fields for the K-stacked 4-mult stages:
            # blocks: [Xr full-128 blocks, Xi full-128 blocks, (Xr tail | Xi tail)]
            NBF = Hc // 128          # full 128-row blocks per component
            REM = Hc - NBF * 128     # tail rows (64 for Hc=320, 0 for 128)
            NBS = 2 * NBF + (1 if REM else 0)
            SW = NBS * Wc
            st1 = [T(work, f"st1_{j}", [128, SW], bf16) for j in (0, 1)]
            st3 = [T(work, f"st3_{j}", [128, SW], bf16) for j in (0, 1)]
            tshift = [T(work, f"tshift_{j}", [128, Wc], bf16) for j in (0, 1)]
            sc16 = [
                [T(work, f"sc16_{k}{j}", [128, FW], bf16) for k in range(4)]
                for j in (0, 1)
            ]
            ac16 = [
                [T(work, f"ac16_{k}{j}", [128, FW], bf16) for k in range(4)]
                for j in (0, 1)
            ]
            # bf16 scratch for reduction products (cheap bf16 reduces)
            prod16 = [T(work, f"prod16_{k}", [128, FW], bf16) for k in range(4)]
            redsink = T(work, "redsink", [128, FW], bf16)
            # bf16 staging for Gauss products (Act evacuates PSUM banks here)
            ksb16 = [
                [T(work, f"ksb16_{k}{j}", [128, FW], bf16) for k in range(3)]
                for j in (0, 1)
            ]

            partials = T(small, "partials", [128, 16])
            redsums = T(small, "redsums", [1, 16])
            asum_t = T(small, "asum", [1, 16])
            scl = T(small, "scl", [1, 16])
            alphas = T(small, "alphas", [1, 8])
            bc = T(small, "bc", [128, 8])
            rr_t = T(small, "rr", [1, 1])
            rrn_t = T(small, "rrn", [1, 1])

            v = nc.vector
            g = nc.gpsimd
            a = nc.scalar
            sp = nc.sync
            STT_OP = "scalar_tensor_tensor"

            # engines are pinned per op class:
            #   DVE: all bf16 elementwise, STT axpys, reduces (fast, flexible)
            #   Pool (GPSIMD): only the q-accumulate f32 adds and a few tail
            #     products -- its flat ~2us/op cost is tolerable off the
            #     critical path (no PSUM access, no TensorScalarPtr there)
            #   Act: all PSUM evacuation copies (fp32 bank -> bf16 SBUF)
            def TTv(out, in0, in1, op):
                v.tensor_tensor(out=out, in0=in0, in1=in1, op=op)

            def TTg(out, in0, in1, op):
                g.tensor_tensor(out=out, in0=in0, in1=in1, op=op)

            def STT(out, in0, scalar, in1, op0, op1):
                v.scalar_tensor_tensor(
                    out=out, in0=in0, scalar=scalar, in1=in1, op0=op0, op1=op1
                )

            def RSUM(out, in_, col):
                v.reduce_sum(out=out[:, col : col + 1], in_=in_,
                             axis=mybir.AxisListType.X)

            # ---------- init ----------

            def load_folded(dst, src2d):
                nbf = Hc // 128
                full = nbf * 128
                if nbf:
                    sp.dma_start(
                        out=dst[:, 0 : nbf * Wc].rearrange("p (b w) -> p b w", b=nbf),
                        in_=src2d[0:full, :].rearrange("(b p) w -> p b w", p=128),
                    )
                if full < Hc:
                    rem = Hc - full
                    sp.dma_start(
                        out=dst[:rem, nbf * Wc : (nbf + 1) * Wc],
                        in_=src2d[full:Hc, :],
                    )

            def store_folded(src, dst2d):
                nbf = Hc // 128
                full = nbf * 128
                if nbf:
                    sp.dma_start(
                        out=dst2d[0:full, :].rearrange("(b p) w -> p b w", p=128),
                        in_=src[:, 0 : nbf * Wc].rearrange("p (b w) -> p b w", b=nbf),
                    )
                if full < Hc:
                    rem = Hc - full
                    sp.dma_start(
                        out=dst2d[full:Hc, :],
                        in_=src[:rem, nbf * Wc : (nbf + 1) * Wc],
                    )

            # 1) zero the junk regions of DMA-target tiles first
            pre = [maskf] + [t for pairt in cs for t in pairt]
            for tl in pre:
                v.memset(tl, 0.0)
            # 2) issue constant + first csm loads (SP) while zeroing the rest
            load_folded(fr, fr_d[:])
            load_folded(fm, fm_d[:])
            load_folded(fd, fd_d[:])
            for gt, gd in ((g1R, g1R_d), (g1I, g1I_d), (g3R, g3R_d),
                           (g3I, g3I_d)):
                sp.dma_start(
                    out=gt[:, : NBS * Wc].rearrange("p (b w) -> p b w", b=NBS),
                    in_=gd[: NBS * 128, :].rearrange("(b p) w -> p b w", p=128),
                )
            load_folded(maskf, mask_d[:])
            sp.dma_start(out=mu_sb[:1, :1], in_=mu_d[None, :])

            def load_csm(ci_, slot):
                load_folded(cs[slot][0], csm_r_d[ci_])
                load_folded(cs[slot][1], csm_i_d[ci_])

            load_csm(0, 0)
            if Cc > 1:
                load_csm(1, 1)
            # fr/fm/fd junk is only ever seen by matmul rhs chunk APs
            # (never read) -- and they are already loaded: do NOT zero them.
            preset = set(id(t) for t in (
                [maskf, fr, fm, fd] + [t for pairt in cs for t in pairt]))
            for tl in zero_me:
                if id(tl) in preset:
                    continue
                if tl.dtype == bf16:
                    v.memset(tl, 0.0)
                else:
                    g.memset(tl, 0.0)
            v.memset(partials, 0.0)
            v.memset(ones_col, 1.0)
            v.memset(ones_row, 1.0)

            # r = us + mu*rec; p0 = r; p16 = bf16(r); b = 0
            # stage us/rec through tiles that are overwritten in iter 0
            load_folded(p_r[1], us[0])
            load_folded(p_i[1], us[1])
            load_folded(q_r, rec[0])
            load_folded(q_i, rec[1])
            psb = psp.tile([128, 16], f32, tag="mm")
            nc.tensor.matmul(
                psb[:, :1], lhsT=ones_row[:1, :128], rhs=mu_sb[:1, :1],
                start=True, stop=True,
            )
            a.copy(out=mu_b[:, :1], in_=psb[:, :1])
            v.scalar_tensor_tensor(out=r_r, in0=q_r, scalar=mu_b[:, :1],
                                   in1=p_r[1], op0=OP.mult, op1=OP.add)
            v.scalar_tensor_tensor(out=r_i, in0=q_i, scalar=mu_b[:, :1],
                                   in1=p_i[1], op0=OP.mult, op1=OP.add)
            a.copy(out=p_r[0], in_=r_r)
            a.copy(out=p_i[0], in_=r_i)
            a.copy(out=p16r, in_=r_r)
            a.copy(out=p16i, in_=r_i)


            def gauss_stage(xr, xi, xs, g1, g2, g3, consume):
                # complex product (xr + i*xi)^T (Gr + i*Gi) via 3 real products:
                # k1 = xs^T g1, k2 = xi^T g2, k3 = xr^T g3
                # Yr = k1 + k2 ; Yi = k1 + k3
                m_order = ([NB - 1] + list(range(NB - 1))) if REM else range(NB)
                for m in m_order:
                    m0, msz = BL[m]
                    k1t = psp.tile([128, Wc], f32, tag="mm")
                    k2t = psp.tile([128, Wc], f32, tag="mm")
                    k3t = psp.tile([128, Wc], f32, tag="mm")
                    # emit k3 first: its input (xr) is ready earliest
                    for bank, (srcd, gg) in ((k3t, (xr, g3)), (k2t, (xi, g2)),
                                             (k1t, (xs, g1))):
                        for k, (k0, ksz) in enumerate(BL):
                            nc.tensor.matmul(
                                bank[:msz, :],
                                lhsT=srcd[:ksz, k * Wc + m0 : k * Wc + m0 + msz],
                                rhs=gg[:ksz, k * Wc : (k + 1) * Wc],
                                start=(k == 0), stop=(k == NB - 1),
                            )
                    consume(m, msz, k1t, k2t, k3t)

            def fourmult_stacked(st, gRst, gIst, consume):
                # K-stacked complex product: the [Xr; Xi] stacking makes the
                # 2K contraction exactly NBS chunks of 128 -- 2 products
                # instead of 4, no padding waste.
                for m, (m0, msz) in enumerate(BL):
                    bR = psp.tile([128, Wc], f32, tag="mm")
                    bI = psp.tile([128, Wc], f32, tag="mm")
                    for bank, gg in ((bR, gRst), (bI, gIst)):
                        for k in range(NBS):
                            nc.tensor.matmul(
                                bank[:msz, :],
                                lhsT=st[:, k * Wc + m0 : k * Wc + m0 + msz],
                                rhs=gg[:, k * Wc : (k + 1) * Wc],
                                start=(k == 0), stop=(k == NBS - 1),
                            )
                    consume(m, msz, bR, bI)

            def evac_copy(dst_r, dst_i):
                def f(m, msz, bR, bI):
                    sl = slice(m * Wc, (m + 1) * Wc)
                    a.copy(out=dst_r[:msz, sl], in_=bR[:msz, :])
                    a.copy(out=dst_i[:msz, sl], in_=bI[:msz, :])
                return f

            def evac_stage_banks(slot):
                # Act copies each Gauss PSUM bank to bf16 staging per m-block
                k1s, k2s, k3s = ksb16[slot]

                def f(m, msz, k1, k2, k3):
                    sl = slice(m * Wc, (m + 1) * Wc)
                    a.copy(out=k1s[:msz, sl], in_=k1[:msz, :])
                    a.copy(out=k2s[:msz, sl], in_=k2[:msz, :])
                    a.copy(out=k3s[:msz, sl], in_=k3[:msz, :])
                return f

            def gauss_tail_combine(slot, st):
                # remainder rows: Yr tail -> stacked tail block p[0:REM];
                # Yi tail -> scratch, then SP-DMA partition-shift to p[REM:]
                if not REM:
                    return
                k1s, k2s, k3s = ksb16[slot]
                csl = slice(NBF * Wc, NBF * Wc + Wc)
                tb = (NBS - 1) * Wc
                TTv(st[:REM, tb : tb + Wc], k1s[:REM, csl], k2s[:REM, csl],
                    OP.add)
                scr = tshift[slot]
                TTv(scr[:REM, :], k1s[:REM, csl], k3s[:REM, csl], OP.add)
                sp.dma_start(out=st[REM : 2 * REM, tb : tb + Wc],
                             in_=scr[:REM, :])

            def gauss_main_combine(slot, st):
                # full blocks: Yr -> stacked blocks [0, NBF), Yi -> [NBF, 2NBF)
                k1s, k2s, k3s = ksb16[slot]
                if NBF:
                    w = NBF * Wc
                    TTv(st[:, 0:w], k1s[:, 0:w], k2s[:, 0:w], OP.add)
                    TTv(st[:, w : 2 * w], k1s[:, 0:w], k3s[:, 0:w], OP.add)

            deferred = []  # per-iteration deferred ops (b axpy), emitted in coils

            def proj_coil(c):
                slot = c % 2
                csr, csi = cs[c % 4]
                cpr, cpi, cps = cp[slot]
                A16, B16, C16, D16 = sc16[slot]
                # projection cp = p * csm (all bf16, DVE)
                TTv(A16, p16r, csr, OP.mult)
                TTv(B16, p16i, csi, OP.mult)
                TTv(cpr, A16, B16, OP.subtract)
                TTv(C16, p16r, csi, OP.mult)
                TTv(D16, p16i, csr, OP.mult)
                TTv(cpi, C16, D16, OP.add)
                TTv(cps, cpr, cpi, OP.add)

            def stage_coil(c, s):
                slot = c % 2
                if s == 0:
                    # FFT rows: Gauss; banks staged by Act; stacked output
                    bk = evac_stage_banks(slot)

                    def con0(m, msz, k1, k2, k3):
                        bk(m, msz, k1, k2, k3)
                        if REM and m == NB - 1:
                            gauss_tail_combine(slot, st1[slot])
                    gauss_stage(*cp[slot], fr, fm, fd, con0)
                    if not REM:
                        gauss_tail_combine(slot, st1[slot])
                    gauss_main_combine(slot, st1[slot])
                elif s == 1:
                    # FFT cols: stacked 4-mult, Act evac, then mask
                    A16, B16 = sc16[slot][0], sc16[slot][1]
                    fourmult_stacked(st1[slot], g1R, g1I, evac_copy(A16, B16))
                    kr, ki, ksm = km[slot]
                    TTv(kr, A16, maskf, OP.mult)
                    TTv(ki, B16, maskf, OP.mult)
                    TTv(ksm, kr, ki, OP.add)
                elif s == 2:
                    # IFFT rows: Gauss (G2/G3 swapped); stacked output
                    bk = evac_stage_banks(slot)

                    def con2(m, msz, k1, k2, k3):
                        bk(m, msz, k1, k2, k3)
                        if REM and m == NB - 1:
                            gauss_tail_combine(slot, st3[slot])
                    gauss_stage(*km[slot], fr, fd, fm, con2)
                    if not REM:
                        gauss_tail_combine(slot, st3[slot])
                    gauss_main_combine(slot, st3[slot])
                else:
                    # IFFT cols: stacked 4-mult straight into z
                    fourmult_stacked(st3[slot], g3R, g3I, evac_copy(*zz[slot]))

            def accum_coil(c, last=0):
                slot = c % 2
                csr, csi = cs[c % 4]
                A16, B16, C16, D16 = ac16[slot]
                zr, zi = zz[slot]
                # q += z * conj(csm): products on DVE; accumulates on Pool
                # normally (hidden under PE). For the tail-adjacent last pair
                # (last=1: first coil, last=2: final coil) split engines so
                # the final q is ready sooner for the reduction dots.
                TTv(A16, zr, csr, OP.mult)
                (TTv if last else TTg)(q_r, q_r, A16, OP.add)
                TTv(B16, zi, csi, OP.mult)
                (TTv if last else TTg)(q_r, q_r, B16, OP.add)
                TTv(C16, zi, csr, OP.mult)
                TTg(q_i, q_i, C16, OP.add)
                TTv(D16, zr, csi, OP.mult)
                TTg(q_i, q_i, D16, OP.subtract)
                # slip one deferred op from the previous iteration's tail in
                if deferred:
                    deferred.pop(0)()

            def coil_pair(it, c0):
                # 2-coil software pipeline: interleave the two coils' stages
                # so one coil's PE matmuls cover the other's evacuation.
                # proj for THIS pair was already emitted by the previous pair
                # (or the iteration prologue); emit the NEXT pair's proj
                # before this pair's q-accumulate so the next pair's first
                # matmuls are never blocked behind accum on DVE.
                pair = [c0] + ([c0 + 1] if c0 + 1 < Cc else [])
                # prefetch the next pair's csm (2 ahead)
                for cn in (c0 + 2, c0 + 3):
                    if cn < Cc:
                        load_csm(cn, cn % 4)
                    elif it + 1 < iters and cn - Cc in (0, 1):
                        load_csm(cn - Cc, (cn - Cc) % 4)
                for s in range(4):
                    for c in pair:
                        stage_coil(c, s)
                for cn in (c0 + 2, c0 + 3):
                    if cn < Cc:
                        proj_coil(cn)
                is_last_pair = c0 + 2 >= Cc
                for j, c in enumerate(pair):
                    accum_coil(c, last=(j + 1 if is_last_pair else 0))

            def reduction_round(k):
                ps1 = psp.tile([1, 16], f32, tag="mm")
                nc.tensor.matmul(ps1[:1, :k], lhsT=ones_col[:, :1],
                                 rhs=partials[:, :k], start=True, stop=True)
                a.copy(out=redsums[:1, :k], in_=ps1[:1, :k])
                din = dram.tile([1, 16], f32, tag="cin")
                dout = dram.tile([1, 16], f32, tag="cout")
                sp.dma_start(out=din[:1, :k], in_=redsums[:1, :k])
                if n_cores > 1 and not no_collective:
                    nc.gpsimd.collective_compute(
                        "AllReduce", OP.add,
                        replica_groups=[list(range(n_cores))],
                        ins=[din[:1, :k].opt()],
                        outs=[dout[:1, :k].opt()],
                    )
                else:
                    sp.dma_start(out=dout[:1, :k], in_=din[:1, :k])
                sp.dma_start(out=asum_t[:1, :k], in_=dout[:1, :k])
                return asum_t

            def dotcol(x, y, col, eng="v"):
                # partials[:, col] = rowsum(bf16(x * y)); the row-sum rides
                # the idle Act engine as a copy with accum_out
                j = dotcol.j
                dotcol.j = (j + 1) % 4
                t = prod16[j]
                (TTv if eng == "v" else TTg)(t, x, y, OP.mult)
                a.activation(
                    out=redsink, in_=t,
                    func=mybir.ActivationFunctionType.Copy,
                    accum_out=partials[:, col : col + 1],
                )
            dotcol.j = 0

            for it in range(iters):
                pcur = it % 2
                pnew = (it + 1) % 2
                pr_, pi_ = p_r[pcur], p_i[pcur]
                # q = mu * p (coils accumulate on top)
                proj_coil(0)
                if Cc > 1:
                    proj_coil(1)
                # q-init and the hoisted (r,r) dots are not needed until the
                # first accumulate -- emit them behind the projections so the
                # first pair's matmuls start as early as possible
                v.tensor_scalar_mul(out=q_r, in0=pr_, scalar1=mu_b[:, :1])
                v.tensor_scalar_mul(out=q_i, in0=pi_, scalar1=mu_b[:, :1])
                if it == 0:
                    dotcol(r_r, r_r, 10, "g")
                    dotcol(r_i, r_i, 11, "v")
                for c0 in range(0, Cc, 2):
                    coil_pair(it, c0)
                # ---- merged reduction round:
                #   pq = sum(q conj(p)); t = sum(q conj(r)); qq = sum(|q|^2)
                #   rr_new = rr - 2 Re(conj(alpha) t) + |alpha|^2 qq
                dotcol(q_r, pr_, 0, "v")
                dotcol(q_r, pi_, 3, "v")
                dotcol(q_r, r_r, 4, "v")
                dotcol(q_r, r_i, 7, "g")
                dotcol(q_r, q_r, 8, "v")
                dotcol(q_i, pi_, 1, "g")
                dotcol(q_i, pr_, 2, "g")
                dotcol(q_i, r_i, 5, "g")
                dotcol(q_i, r_r, 6, "g")
                dotcol(q_i, q_i, 9, "v")
                k = 12 if it == 0 else 10
                asum = reduction_round(k)
                TTv(out=scl[:1, 0:1], in0=asum[:1, 0:1], in1=asum[:1, 1:2],
                    op=OP.add)       # pq_r
                TTv(out=scl[:1, 1:2], in0=asum[:1, 2:3], in1=asum[:1, 3:4],
                    op=OP.subtract)  # pq_i
                TTv(out=scl[:1, 6:7], in0=asum[:1, 4:5], in1=asum[:1, 5:6],
                    op=OP.add)       # t_r
                TTv(out=scl[:1, 7:8], in0=asum[:1, 6:7], in1=asum[:1, 7:8],
                    op=OP.subtract)  # t_i
                TTv(out=scl[:1, 8:9], in0=asum[:1, 8:9], in1=asum[:1, 9:10],
                    op=OP.add)       # qq
                if it == 0:
                    TTv(out=rr_t[:1, :1], in0=asum[:1, 10:11],
                        in1=asum[:1, 11:12], op=OP.add)
                TTv(out=scl[:1, 2:3], in0=scl[:1, 0:1], in1=scl[:1, 0:1],
                    op=OP.mult)
                TTv(out=scl[:1, 3:4], in0=scl[:1, 1:2], in1=scl[:1, 1:2],
                    op=OP.mult)
                TTv(out=scl[:1, 2:3], in0=scl[:1, 2:3], in1=scl[:1, 3:4],
                    op=OP.add)       # |pq|^2
                v.reciprocal(out=scl[:1, 5:6], in_=scl[:1, 2:3])
                TTv(out=scl[:1, 4:5], in0=rr_t[:1, :1], in1=scl[:1, 5:6],
                    op=OP.mult)      # g = rr/|pq|^2
                # alphas: [a_r, na_i, na_r, a_i, beta]; alpha = g*conj(pq)
                TTv(out=alphas[:1, 0:1], in0=scl[:1, 4:5], in1=scl[:1, 0:1],
                    op=OP.mult)
                TTv(out=alphas[:1, 1:2], in0=scl[:1, 4:5], in1=scl[:1, 1:2],
                    op=OP.mult)
                v.tensor_scalar_mul(out=alphas[:1, 2:3], in0=alphas[:1, 0:1],
                                    scalar1=-1.0)
                v.tensor_scalar_mul(out=alphas[:1, 3:4], in0=alphas[:1, 1:2],
                                    scalar1=-1.0)
                # broadcast alpha immediately: the r-updates only need it
                v.reciprocal(out=scl[:1, 14:15], in_=rr_t[:1, :1])
                psbA = psp.tile([128, 16], f32, tag="mm")
                nc.tensor.matmul(psbA[:, :4], lhsT=ones_row[:1, :128],
                                 rhs=alphas[:1, :4], start=True, stop=True)
                a.copy(out=bc[:, :4], in_=psbA[:, :4])
                a_r = bc[:, 0:1]
                na_i = bc[:, 1:2]
                na_r = bc[:, 2:3]
                a_i = bc[:, 3:4]
                bet = bc[:, 4:5]
                # critical path: r -= alpha*q on DVE, while the beta chain
                # (rr_new expansion) runs concurrently on GPSIMD
                v.scalar_tensor_tensor(out=r_r, in0=q_r, scalar=na_r, in1=r_r,
                                       op0=OP.mult, op1=OP.add)
                TTg(scl[:1, 9:10], alphas[:1, 0:1], scl[:1, 6:7], OP.mult)
                TTg(scl[:1, 10:11], alphas[:1, 1:2], scl[:1, 7:8], OP.mult)
                TTg(scl[:1, 9:10], scl[:1, 9:10], scl[:1, 10:11], OP.add)
                TTg(scl[:1, 11:12], alphas[:1, 0:1], alphas[:1, 0:1], OP.mult)
                TTg(scl[:1, 12:13], alphas[:1, 1:2], alphas[:1, 1:2], OP.mult)
                TTg(scl[:1, 11:12], scl[:1, 11:12], scl[:1, 12:13], OP.add)
                TTg(scl[:1, 12:13], scl[:1, 11:12], scl[:1, 8:9], OP.mult)
                TTg(scl[:1, 10:11], scl[:1, 9:10], scl[:1, 9:10], OP.add)
                TTg(scl[:1, 13:14], rr_t[:1, :1], scl[:1, 10:11], OP.subtract)
                TTg(rrn_t[:1, :1], scl[:1, 13:14], scl[:1, 12:13], OP.add)
                TTg(alphas[:1, 4:5], rrn_t[:1, :1], scl[:1, 14:15], OP.mult)
                psbB = psp.tile([128, 16], f32, tag="mm")
                nc.tensor.matmul(psbB[:, :1], lhsT=ones_row[:1, :128],
                                 rhs=alphas[:1, 4:5], start=True, stop=True)
                a.copy(out=bc[:, 4:5], in_=psbB[:, :1])
                a.copy(out=rr_t[:1, :1], in_=rrn_t[:1, :1])
                v.scalar_tensor_tensor(out=r_i, in0=q_i, scalar=na_r, in1=r_i,
                                       op0=OP.mult, op1=OP.add)
                v.scalar_tensor_tensor(out=r_r, in0=q_i, scalar=a_i, in1=r_r,
                                       op0=OP.mult, op1=OP.add)
                v.scalar_tensor_tensor(out=r_i, in0=q_r, scalar=na_i, in1=r_i,
                                       op0=OP.mult, op1=OP.add)
                v.scalar_tensor_tensor(out=p_r[pnew], in0=pr_, scalar=bet,
                                       in1=r_r, op0=OP.mult, op1=OP.add)
                v.scalar_tensor_tensor(out=p_i[pnew], in0=pi_, scalar=bet,
                                       in1=r_i, op0=OP.mult, op1=OP.add)
                a.copy(out=p16r, in_=p_r[pnew])
                a.copy(out=p16i, in_=p_i[pnew])

                # b += alpha*p (old p) -- deferred into next iteration's coils
                def mk(eng, out, in0, sca, in1):
                    def run():
                        getattr(eng, STT_OP)(out=out, in0=in0, scalar=sca,
                                             in1=in1, op0=OP.mult, op1=OP.add)
                    return run

                dops = [
                    mk(v, b_r, pr_, a_r, b_r),
                    mk(v, b_i, pi_, a_r, b_i),
                    mk(v, b_r, pi_, na_i, b_r),
                    mk(v, b_i, pr_, a_i, b_i),
                ]
                if it + 1 < iters:
                    deferred.extend(dops)
                else:
                    for d in dops:
                        d()

            import os as _os
            if _os.environ.get("KDBG") == "q":
                store_folded(q_r, out_d[0])
                store_folded(q_i, out_d[1])
            elif _os.environ.get("KDBG") == "z":
                zlast = zz[(Cc - 1) % 2]
                v.tensor_scalar_mul(out=r_r, in0=zlast[0], scalar1=1.0)
                v.tensor_scalar_mul(out=r_i, in0=zlast[1], scalar1=1.0)
                store_folded(r_r, out_d[0])
                store_folded(r_i, out_d[1])
            elif _os.environ.get("KDBG") == "km":
                klast = km[(Cc - 1) % 2]
                v.tensor_scalar_mul(out=r_r, in0=klast[0], scalar1=1.0)
                v.tensor_scalar_mul(out=r_i, in0=klast[1], scalar1=1.0)
                store_folded(r_r, out_d[0])
                store_folded(r_i, out_d[1])
            elif _os.environ.get("KDBG") == "s1":
                stl = st1[(Cc - 1) % 2]
                wv = NBF * Wc
                # Yr: full blocks then tail block lower half
                v.tensor_scalar_mul(out=r_r[:, 0:wv], in0=stl[:, 0:wv],
                                    scalar1=1.0)
                v.tensor_scalar_mul(out=r_i[:, 0:wv], in0=stl[:, wv : 2 * wv],
                                    scalar1=1.0)
                if REM:
                    tbv = (NBS - 1) * Wc
                    v.tensor_scalar_mul(
                        out=r_r[:REM, NBF * Wc : NBF * Wc + Wc],
                        in0=stl[:REM, tbv : tbv + Wc], scalar1=1.0)
                    v.tensor_scalar_mul(
                        out=r_i[:REM, NBF * Wc : NBF * Wc + Wc],
                        in0=stl[REM : 2 * REM, tbv : tbv + Wc], scalar1=1.0)
                store_folded(r_r, out_d[0])
                store_folded(r_i, out_d[1])
            elif _os.environ.get("KDBG") == "p0":
                store_folded(p_r[0], out_d[0])
                store_folded(p_i[0], out_d[1])
            elif _os.environ.get("KDBG") == "mub":
                nc.scalar.copy(out=r_r[:, 0:1], in_=mu_b[:, 0:1])
                store_folded(r_r, out_d[0])
                store_folded(r_i, out_d[1])
            elif _os.environ.get("KDBG") == "cp":
                clast = cp[(Cc - 1) % 2]
                v.tensor_scalar_mul(out=r_r, in0=clast[0], scalar1=1.0)
                v.tensor_scalar_mul(out=r_i, in0=clast[1], scalar1=1.0)
                store_folded(r_r, out_d[0])
                store_folded(r_i, out_d[1])
            else:
                store_folded(b_r, out_d[0])
                store_folded(b_i, out_d[1])

    nc.compile()
    return nc


def _stack_g(GA, GB, Hc):
    # row order must match the on-chip stacked layout: full 128-row blocks of
    # A, then of B, then the interleaved tail block [A-tail; B-tail]
    full = (Hc // 128) * 128
    return np.concatenate([GA[:full], GB[:full], GA[full:], GB[full:]], axis=0)


def _prep_consts(Hc):
    import ml_dtypes

    bf = ml_dtypes.bfloat16
    Fc = _centered_dft(Hc)
    fr = np.ascontiguousarray(Fc.real).astype(np.float32)
    fi = np.ascontiguousarray(Fc.imag).astype(np.float32)
    fni = -fi
    return {
        "f_r": fr.astype(bf),
        "f_m": (-(fr + fi)).astype(bf),
        "f_d": (fi - fr).astype(bf),
        # stage 1 (FFT):  Yr = Xr^T fr + Xi^T (-fi); Yi = Xr^T fi + Xi^T fr
        "g1R": _stack_g(fr, fni, Hc).astype(bf),
        "g1I": _stack_g(fi, fr, Hc).astype(bf),
        # stage 3 (IFFT): Yr = Xr^T fr + Xi^T fi;    Yi = Xr^T (-fi) + Xi^T fr
        "g3R": _stack_g(fr, fi, Hc).astype(bf),
        "g3I": _stack_g(fni, fr, Hc).astype(bf),
    }


def kernel(us_image, reconstruction, mask, csm_r, csm_i, mu):
    global LAST_RESULT
    import ml_dtypes
    from concourse.bass_utils import run_bass_kernel_spmd

    bf = ml_dtypes.bfloat16
    Bc, _, Hc, Wc = us_image.shape
    Cc = csm_r.shape[1]
    n_cores = Bc
    iters = CG_ITER

    key = (Hc, Wc, Cc, iters, n_cores)
    if key not in _nc_cache:
        _nc_cache[key] = _build(Hc, Wc, Cc, iters, n_cores)
    nc = _nc_cache[key]

    gconsts = _prep_consts(Hc)

    in_maps = []
    for b in range(n_cores):
        in_maps.append(
            {
                "us_image": np.ascontiguousarray(us_image[b], dtype=np.float32),
                "reconstruction": np.ascontiguousarray(
                    reconstruction[b], dtype=np.float32
                ),
                "mask": np.ascontiguousarray(mask[b, 0]).astype(bf),
                "csm_r": np.ascontiguousarray(csm_r[b]).astype(bf),
                "csm_i": np.ascontiguousarray(csm_i[b]).astype(bf),
                "mu": np.ascontiguousarray(mu, dtype=np.float32),
                **gconsts,
            }
        )

    res = run_bass_kernel_spmd(nc, in_maps, core_ids=list(range(n_cores)))
    LAST_RESULT = res
    out = np.stack([res.results[b]["out"] for b in range(n_cores)], axis=0)
    return out.astype(np.float32)
